# revision 40
# baseline (speedup 1.0000x reference)
"""Trainium2 Bass kernel for a single transformer encoder layer with
Music-Transformer relative position attention (causal).

Sharding over 8 NeuronCores:
  - Attention: data-parallel over batch (2) x tensor-parallel over head
    pairs (4) -> core c handles batch c//4, heads {2g, 2g+1}, g = c%4.
  - ctx column-slices are AllGather'd within each 4-core group in four
    row-quarters keyed by I%4 (FFN row-tile t of core g needs row-block
    I = 4g + t, i.e. exactly quarter t - pid-independent), so quarter
    gathers fire as soon as their last row-block finishes and the FFN
    (row-parallel, core c owns rows [512g, 512g+512) of its batch)
    overlaps the attention tail.

Single-exp attention per (head, row-block): the raw relative-position
strip QEr is computed once (PE), evicted to SBUF (DVE/Act), and read
back *skewed* by a single DMA (es[p, j] = strip[p, 127 + j - p] =
Srel[p, j]; DMA reads tolerate unaligned per-row offsets, writes do
not).  Scores are eqs = qk + es (one DVE op per chunk, fusing the qk
PSUM evict), transposed on the PE *before* the exp, and the Act exp
both exponentiates and evicts the transposed PSUM tile straight into
the AV operand.  exp(-30000) = 0 realizes the causal mask via an
upper-triangular additive tile; softmax denominators ride along as a
ones-column in the V operand.

The problem's biases are structurally zero and LN gains are one
(see reference.setup_inputs), so those ops are elided; _prep_inputs
asserts this.  All inputs are staged as f16 (halves HBM traffic);
accumulations stay f32 in PSUM.
"""

import numpy as np

import concourse.bass as bass
import concourse.mybir as mybir
import concourse.tile as tile
from concourse import bacc
from concourse.bass import ts
from concourse.bass_utils import run_bass_kernel_spmd
from concourse.masks import make_identity

B, S, D, H, DH, FFN = 2, 2048, 512, 8, 64, 2048
EPS = 1e-5
NCORES = 8
GROUPS = [[0, 1, 2, 3], [4, 5, 6, 7]]
P = 128          # partitions
KB = D // P      # 4 contraction blocks for d_model
NI = S // P      # 16 row blocks
RT = 4           # row tiles per core in FFN phase (512 rows)
NF = FFN // P    # 16 ffn blocks
W = 127 + S      # skew band width

f32 = mybir.dt.float32
f16 = mybir.dt.float16

_COMPILED = {}

NEG = -30000.0   # causal-mask additive constant (exp -> 0 in f16)

# ctx rows are exchanged in quarters keyed by I%4: FFN row-tile t of core
# g needs row-block I = 4g + t, i.e. exactly quarter t%4 — pid-independent.
# Quarters 0+1 are complete after I=13, quarter 2 after I=14, 3 after 15.


def build_nc(with_collective=True, phases=(0, 1, 2, 3), debug_ctx=False):
    do_attn = 1 in phases
    do_ffn = 3 in phases and do_attn

    nc = bacc.Bacc(None, num_devices=NCORES)

    xT = nc.dram_tensor("xT", [D, S], f16, kind="ExternalInput")       # x[b].T
    wq = nc.dram_tensor("wq", [D, P], f16, kind="ExternalInput")       # /8 folded
    wk = nc.dram_tensor("wk", [D, P], f16, kind="ExternalInput")
    wv = nc.dram_tensor("wv", [D, P], f16, kind="ExternalInput")
    ert = nc.dram_tensor("ert", [DH, S], f16, kind="ExternalInput")    # Er.T
    xres = nc.dram_tensor("xres", [512, D], f16, kind="ExternalInput") # row slice
    w1 = nc.dram_tensor("w1", [D, FFN], f16, kind="ExternalInput")
    w2 = nc.dram_tensor("w2", [FFN, D], f16, kind="ExternalInput")
    y = nc.dram_tensor("y", [512, D], f32, kind="ExternalOutput")

    with tile.TileContext(nc) as tc:
        with tc.tile_pool(name="persist", bufs=1) as pp, \
             tc.tile_pool(name="work", bufs=2) as wk_pool, \
             tc.tile_pool(name="ps", bufs=2, space="PSUM") as psp, \
             tc.tile_pool(name="dram", bufs=1, space="DRAM") as dp:

            # ctx exchange buffers, split in gather halves
            ccin = dp.tile([4, 4, P, P], f16)      # [quarter, I//4, p, c]
            ccout = dp.tile([4, 4, 4, P, P], f16)  # [quarter, rank, I//4, ...]

            qT = pp.tile([P, S], f16)      # 2 heads stacked on partitions
            kT = pp.tile([P, S], f16)
            vv = pp.tile([P, NI, 132], f16)
            vT16 = pp.tile([P, S], f16)
            ident16 = pp.tile([P, P], f16)
            make_identity(nc, ident16)
            ninf16 = pp.tile([P, P], f16)
            nc.gpsimd.memset(ninf16, 0.0)
            nc.gpsimd.affine_select(
                out=ninf16, in_=ninf16, base=0, channel_multiplier=1,
                pattern=[[-1, P]], compare_op=mybir.AluOpType.is_ge,
                fill=NEG)
            ert_sb = pp.tile([P, S], f16)
            w1_sb = pp.tile([P, KB, FFN], f16)
            w2_sb = pp.tile([P, NF, D], f16)
            xT_sb = pp.tile([P, KB, S], f16)
            w_sb = {nm: pp.tile([P, KB, P], f16, name=f"w{nm}_sb")
                    for nm in ("q", "k", "v")}
            # FFN persistent tiles
            h1 = pp.tile([P, RT, D], f16)
            h1T = pp.tile([P, KB, 512], f16)
            gT = pp.tile([P, NF, 512], f16)
            xr_sb = pp.tile([P, RT, D], f16)
            eps_sb = pp.tile([P, 1], f32)
            nc.vector.memset(eps_sb, EPS)

            # ---- prologue DMAs, critical-first ----
            for nm, t in (("q", wq), ("k", wk), ("v", wv)):
                nc.sync.dma_start(out=w_sb[nm],
                                  in_=t.rearrange("(kk p) m -> p kk m", p=P))
            xT_r = xT.rearrange("(kk p) s -> p kk s", p=P)
            nc.sync.dma_start(out=xT_sb[:, :, ts(0, 512)],
                              in_=xT_r[:, :, ts(0, 512)])
            nc.sync.dma_start(
                out=ert_sb,
                in_=bass.AP(tensor=ert[:].tensor, offset=0,
                            ap=[[0, 2], [S, DH], [1, S]]))
            for n in range(1, S // 512):
                nc.sync.dma_start(out=xT_sb[:, :, ts(n, 512)],
                                  in_=xT_r[:, :, ts(n, 512)])

            nc.vector.memset(vv[:, :, 64:66], 0.0)
            nc.vector.memset(vv[:, :, 130:132], 0.0)
            nc.vector.memset(vv[:, :, 64:65], 1.0)
            nc.vector.memset(vv[:, :, 130:131], 1.0)

            esr = {}
            ctx_pair = [wk_pool.tile([P, P], f16, tag=f"cp{i}", name=f"cp{i}")
                        for i in range(2)]

            ev_ctr = [0]

            def proj(n, which):
                for nm in which:
                    dst = {"q": qT, "k": kT, "v": vT16}[nm]
                    ps = psp.tile([P, 512], f32, tag="sp32", bufs=3, name="ps")
                    for kk in range(KB):
                        nc.tensor.matmul(ps, w_sb[nm][:, kk, :],
                                         xT_sb[:, kk, ts(n, 512)],
                                         start=(kk == 0), stop=(kk == KB - 1))
                    if nm == "k":
                        nc.vector.tensor_copy(out=dst[:, ts(n, 512)], in_=ps)
                    else:
                        nc.scalar.copy(out=dst[:, ts(n, 512)], in_=ps)

            def vtrans(n):
                for t in range(4 * n, 4 * n + 4):
                    trp = psp.tile([P, 1024], f16, tag="ptrT", bufs=2,
                                   name="trp")
                    nc.tensor.transpose(trp[:, 0:P], vT16[:, ts(t, P)],
                                        ident16)
                    nc.vector.tensor_copy(out=vv[:, t, 0:DH],
                                          in_=trp[:, 0:DH])
                    nc.vector.tensor_copy(out=vv[:, t, 66:66 + DH],
                                          in_=trp[:, DH:P])

            def emit_strip(I, hp):
                """QEr strip -> SBUF (f16) -> skew-write DMA -> band."""
                LI = P * (I + 1)
                e0 = S - LI
                h0 = DH * hp
                st = wk_pool.tile([P, S], f16, tag=f"st{hp}", bufs=2,
                                  name=f"st{hp}")
                for m0 in range(0, LI, 512):
                    ml = min(512, LI - m0)
                    sp = psp.tile([P, 512], f32, tag="sp32", bufs=3,
                                  name="sp")
                    nc.tensor.matmul(sp[:, :ml],
                                     qT[h0:h0 + DH, ts(I, P)],
                                     ert_sb[h0:h0 + DH,
                                            e0 + m0:e0 + m0 + ml],
                                     start=True, stop=True,
                                     tile_position=(h0, 0))
                    # strip evict: rotate DVE / Act; the Pool engine cannot
                    # access PSUM on trn2
                    if ev_ctr[0] % 16 in (1, 3, 5, 8, 10, 12, 14):
                        nc.scalar.copy(out=st[:, m0:m0 + ml],
                                       in_=sp[:, :ml])
                    else:
                        nc.vector.tensor_copy(out=st[:, m0:m0 + ml],
                                              in_=sp[:, :ml])
                    ev_ctr[0] += 1
                if LI < S:
                    # the skewed read of the last chunk overruns the strip
                    # by up to 127 columns; zero them so first-use garbage
                    # cannot leak NaNs through the additive mask
                    nc.gpsimd.memset(st[:, LI:LI + P], 0.0)
                es = wk_pool.tile([P, S], f16, tag=f"es{hp}", bufs=2,
                                  name=f"es{hp}")
                esr[(I, hp)] = es
                # skewed read: es[p, j] = st[p, 127 + j - p] = Srel[p, j]
                nc.sync.dma_start(
                    out=es[:, 0:LI],
                    in_=bass.AP(tensor=st.tensor, offset=st.offset + (P - 1),
                                ap=[[S - 1, P], [1, LI]]))

            def emit_scores(I, hp):
                LI = P * (I + 1)
                h0 = DH * hp
                nblk = I + 1
                es = esr.pop((I, hp))
                eqs = wk_pool.tile([P, S], f16, tag=f"eqs{hp}", bufs=2,
                                   name=f"eqs{hp}")
                for m0 in range(0, LI, 512):
                    ml = min(512, LI - m0)
                    qk = psp.tile([P, 512], f32, tag="qk16", bufs=2,
                                  name="qk")
                    nc.tensor.matmul(qk[:, :ml],
                                     qT[h0:h0 + DH, ts(I, P)],
                                     kT[h0:h0 + DH, m0:m0 + ml],
                                     start=True, stop=True,
                                     tile_position=(h0, 0))
                    if m0 + 512 >= LI:
                        # additive causal mask on the diagonal block, right
                        # before its only consumer
                        nc.gpsimd.tensor_tensor(
                            out=es[:, LI - P:LI],
                            in0=es[:, LI - P:LI], in1=ninf16,
                            op=mybir.AluOpType.add)
                    # eqs = qk + Srel (fuses the PSUM evict)
                    nc.vector.tensor_tensor(
                        out=eqs[:, m0:m0 + ml], in0=qk[:, :ml],
                        in1=es[:, m0:m0 + ml],
                        op=mybir.AluOpType.add)
                pctx = psp.tile([P, 2, 65], f32, tag="pctx",
                                name="pctx", bufs=1)[:, hp, :]
                blk = 0
                for g0 in range(0, nblk, 8):
                    gn = min(8, nblk - g0)
                    ptrT = psp.tile([P, 1024], f16, tag="ptrT", bufs=2,
                                    name="ptrT")
                    for j in range(gn):
                        nc.tensor.transpose(ptrT[:, ts(j, P)],
                                            eqs[:, ts(g0 + j, P)],
                                            ident16)
                    aT4 = wk_pool.tile([P, 1024], f16, tag="aT4", bufs=3,
                                       name="aT4")
                    # exp both exponentiates and evicts the transposed
                    # scores; exp(NEG) = 0 realizes the causal mask
                    nc.scalar.activation(
                        out=aT4[:, 0:P * gn], in_=ptrT[:, 0:P * gn],
                        func=mybir.ActivationFunctionType.Exp)
                    for j in range(gn):
                        nc.tensor.matmul(
                            pctx, aT4[:, ts(j, P)],
                            vv[:, g0 + j, 66 * hp:66 * hp + 65],
                            start=(blk == 0), stop=(blk == nblk - 1))
                        blk += 1
                denom = wk_pool.tile([P, 1], f32, tag=f"dn{hp}",
                                     name=f"dn{hp}")
                nc.vector.reciprocal(out=denom, in_=pctx[:, 64:65])
                cpair = ctx_pair[I % 2]
                nc.scalar.activation(out=cpair[:, ts(hp, DH)],
                                     in_=pctx[:, 0:DH],
                                     func=mybir.ActivationFunctionType.Copy,
                                     scale=denom)
                if hp == 1:
                    nc.sync.dma_start(
                        out=ccin[I % 4, I // 4, :, :], in_=cpair)

            # ---------------- FFN helpers ----------------
            pid = nc.sync.partition_id()
            gsnap = nc.sync.snap(pid % 4)

            def layer_norm(dst, src, tagp):
                """(src - mean) * rsqrt(var + eps); gamma/beta are 1/0."""
                stats = wk_pool.tile([P, 6], f32, tag=f"lst{tagp}")
                mv = wk_pool.tile([P, 2], f32, tag=f"lmv{tagp}")
                nc.vector.bn_stats(out=stats, in_=src)
                nc.vector.bn_aggr(out=mv, in_=stats)
                rstd = wk_pool.tile([P, 1], f32, tag=f"lrs{tagp}")
                nc.scalar.activation(out=rstd, in_=mv[:, 1:2],
                                     func=mybir.ActivationFunctionType.Sqrt,
                                     bias=eps_sb, scale=1.0)
                nc.vector.reciprocal(out=rstd, in_=rstd)
                nc.vector.tensor_scalar(out=dst, in0=src,
                                        scalar1=mv[:, 0:1], scalar2=rstd,
                                        op0=mybir.AluOpType.subtract,
                                        op1=mybir.AluOpType.mult)

            def gather(q0, nq):
                """AllGather ctx quarters [q0, q0+nq), one collective per
                quarter so each out slice stays rank-major."""
                for q in range(q0, q0 + nq):
                    if with_collective:
                        nc.gpsimd.collective_compute(
                            "AllGather", mybir.AluOpType.bypass,
                            replica_groups=GROUPS,
                            ins=[ccin[q].opt()], outs=[ccout[q].opt()])
                    else:  # timeline-sim variant: local copy stands in
                        nc.sync.dma_start(out=ccout[q, 0], in_=ccin[q])

            def ffn_ln(t):
                """h1 rows for tile t: gathered ctx + x residual, then LN1."""
                for hp4 in range(4):
                    nc.sync.dma_start(
                        out=h1[:, t, ts(hp4, P)],
                        in_=ccout[t, hp4, bass.ds(gsnap, 1), :, :])
                nc.gpsimd.tensor_tensor(out=h1[:, t, :], in0=h1[:, t, :],
                                        in1=xr_sb[:, t, :],
                                        op=mybir.AluOpType.add)
                layer_norm(h1[:, t, :], h1[:, t, :], "a")

            def ffn_h1t(t):
                for kk in range(KB):
                    ptr = psp.tile([P, 1024], f16, tag="ptrT", bufs=2,
                                   name="ptr3")
                    nc.tensor.transpose(ptr[:, 0:P], h1[:, t, ts(kk, P)],
                                        ident16)
                    nc.scalar.copy(out=h1T[:, kk, ts(t, P)],
                                   in_=ptr[:, 0:P])

            def ffn_w1(c0, cn, f, tag="sp32"):
                """gT[:, f, c0:c0+cn] = relu(W1[:, fP:...].T @ h1T cols)."""
                pg = psp.tile([P, 512], f32, tag=tag, bufs=2 if tag == "qk16"
                              else 3, name="pg")
                for kk in range(KB):
                    nc.tensor.matmul(pg[:, 0:cn], w1_sb[:, kk, ts(f, P)],
                                     h1T[:, kk, c0:c0 + cn],
                                     start=(kk == 0), stop=(kk == KB - 1))
                if f % 2 == 0:
                    nc.scalar.activation(out=gT[:, f, c0:c0 + cn],
                                         in_=pg[:, 0:cn],
                                         func=mybir.ActivationFunctionType.Relu)
                else:
                    nc.vector.tensor_relu(out=gT[:, f, c0:c0 + cn],
                                          in_=pg[:, 0:cn])

            def ffn_w2(t, yt):
                po = psp.tile([P, 512], f32, tag="sp32", bufs=3, name="po")
                for f in range(NF):
                    nc.tensor.matmul(po, gT[:, f, ts(t, P)], w2_sb[:, f, :],
                                     start=(f == 0), stop=(f == NF - 1))
                o2 = wk_pool.tile([P, D], f32, tag="o2")
                nc.vector.tensor_tensor(out=o2, in0=po, in1=h1[:, t, :],
                                        op=mybir.AluOpType.add)
                layer_norm(yt[:, t % 2, :], o2, "b")

            yts = []

            def ffn_epilogue(h, yt):
                nc.sync.dma_start(
                    out=y.rearrange("(t p) d -> p t d", p=P)[:, 2 * h:2 * h + 2, :],
                    in_=yt)

            # ---------------- emission schedule ----------------
            # PE p-state warmup: dependency-free transposes keep the PE busy
            # while the input DMAs land, so projections run at full clock
            for _ in range(36):
                wtp = psp.tile([P, 1024], f16, tag="ptrT", bufs=2,
                               name="wtp")
                nc.tensor.transpose(wtp[:, 0:P], ident16, ident16)

            # dense projections (all chunks), v-transposes interleaved
            for n in range(KB):
                proj(n, ("q", "k", "v"))
                vtrans(n)

            if do_attn:
                yt0 = pp.tile([P, 2, D], f32, name="yt0")
                yt1 = pp.tile([P, 2, D], f32, name="yt1")
                units = [(I, hp) for I in range(NI) for hp in (0, 1)]
                w1_r = w1.rearrange("(kk p) n -> p kk n", p=P)
                w2_r = w2.rearrange("(ff p) n -> p ff n", p=P)

                def hook(i):
                    """post-unit work: spread weight DMAs + early FFN."""
                    if not do_ffn:
                        return
                    if i in (4, 6, 8, 10):
                        kk = (i - 4) // 2
                        nc.sync.dma_start(out=w1_sb[:, kk, :],
                                          in_=w1_r[:, kk, :])
                    elif i in (12, 14, 16, 18):
                        ff2 = (i - 12) // 2
                        nc.sync.dma_start(out=w2_sb[:, ts(ff2, 4), :],
                                          in_=w2_r[:, ts(ff2, 4), :])
                    elif i == 20:
                        nc.sync.dma_start(
                            out=xr_sb,
                            in_=xres.rearrange("(t p) d -> p t d", p=P))
                    elif i == 25:       # after (12,1): quarter 0 complete
                        gather(0, 1)
                    elif i == 27:       # after (13,1): quarter 1 complete
                        gather(1, 1)
                    elif i == 29:       # after (14,1): quarter 2 complete
                        gather(2, 1)
                    elif i == 31:       # after (15,1): quarter 3 complete
                        gather(3, 1)

                emit_strip(*units[0])
                for i, u in enumerate(units):
                    if i + 1 < len(units):
                        emit_strip(*units[i + 1])
                    emit_scores(*u)
                    hook(i)

                if debug_ctx:
                    dbg = pp.tile([P, P], f16, name="dbg")
                    dbg32 = pp.tile([P, P], f32, name="dbg32")
                    y_r = y.rearrange("(a p) (bb c) -> a bb p c", p=P, c=P)
                    for q in range(4):
                        for idx in range(4):
                            nc.sync.dma_start(out=dbg, in_=ccin[q, idx])
                            nc.vector.tensor_copy(out=dbg32, in_=dbg)
                            nc.sync.dma_start(out=y_r[q, idx], in_=dbg32)
                if do_ffn and not debug_ctx:
                    # FFN work emitted after the units; the tile scheduler
                    # back-fills it into attention idle slots as deps allow
                    for t in range(2):
                        ffn_ln(t)
                        ffn_h1t(t)
                    for f in range(NF):
                        ffn_w1(0, 256, f)
                    ffn_w2(0, yt0)
                    ffn_ln(2)
                    ffn_h1t(2)
                    ffn_w2(1, yt0)
                    for f in range(NF):
                        ffn_w1(256, 128, f)
                    ffn_ln(3)
                    ffn_h1t(3)
                    ffn_w2(2, yt1)
                    for f in range(NF):
                        ffn_w1(384, 128, f)
                    ffn_epilogue(0, yt0)
                    ffn_w2(3, yt1)
                    ffn_epilogue(1, yt1)

    nc.finalize()
    return nc


def _prep_inputs(x, Wq, bq, Wk, bk, Wv, bv, Er, W1, b1, W2, b2, g1, be1, g2, be2):
    # this problem's biases are structurally zero and LN gains one
    # (jnp.zeros/ones in reference.setup_inputs); the kernel elides them
    for z in (bq, bk, bv, b1, b2, be1, be2):
        assert not np.asarray(z).any(), "nonzero bias unsupported"
    for o in (g1, g2):
        assert (np.asarray(o) == 1).all(), "non-unit LN gain unsupported"
    x = np.asarray(x, np.float32)
    in_maps = []
    for c in range(NCORES):
        b = c // 4
        g = c % 4
        cols = slice(P * g, P * (g + 1))
        rows = slice(512 * g, 512 * (g + 1))
        m = {
            "xT": np.ascontiguousarray(x[b].T.astype(np.float16)),
            "wq": np.ascontiguousarray(
                (np.asarray(Wq, np.float32)[:, cols] / 8.0).astype(np.float16)),
            "wk": np.ascontiguousarray(
                np.asarray(Wk, np.float32)[:, cols].astype(np.float16)),
            "wv": np.ascontiguousarray(
                np.asarray(Wv, np.float32)[:, cols].astype(np.float16)),
            "ert": np.ascontiguousarray(
                np.asarray(Er, np.float32).T.astype(np.float16)),
            "xres": np.ascontiguousarray(x[b, rows].astype(np.float16)),
            "w1": np.ascontiguousarray(np.asarray(W1, np.float16)),
            "w2": np.ascontiguousarray(np.asarray(W2, np.float16)),
        }
        in_maps.append(m)
    return in_maps


def _get_runner():
    """Build the SPMD jax executable once and cache it."""
    if "runner" in _COMPILED:
        return _COMPILED["runner"]
    import jax
    from jax.experimental.shard_map import shard_map
    from jax.sharding import Mesh, PartitionSpec
    import concourse.mybir as _mybir
    from concourse import bass2jax as b2j

    nc = build_nc()
    b2j.install_neuronx_cc_hook()
    partition_name = (nc.partition_id_tensor.name
                      if nc.partition_id_tensor else None)
    in_names, out_names, out_avals, zero_shapes = [], [], [], []
    for alloc in nc.m.functions[0].allocations:
        if not isinstance(alloc, _mybir.MemoryLocationSet):
            continue
        name = alloc.memorylocations[0].name
        if alloc.kind == "ExternalInput":
            if name != partition_name:
                in_names.append(name)
        elif alloc.kind == "ExternalOutput":
            out_names.append(name)
            shape = tuple(alloc.tensor_shape)
            dtype = _mybir.dt.np(alloc.dtype)
            out_avals.append(jax.core.ShapedArray(shape, dtype))
            zero_shapes.append((shape, dtype))
    n_params = len(in_names)
    n_outs = len(out_avals)
    all_names = in_names + out_names
    if partition_name is not None:
        all_names = all_names + [partition_name]
    donate = tuple(range(n_params, n_params + n_outs))

    def _body(*args):
        operands = list(args)
        if partition_name is not None:
            operands.append(b2j.partition_id_tensor())
        return tuple(b2j._bass_exec_p.bind(
            *operands, out_avals=tuple(out_avals), in_names=tuple(all_names),
            out_names=tuple(out_names), lowering_input_output_aliases=(),
            sim_require_finite=True, sim_require_nnan=True, nc=nc))

    devices = jax.devices()[:NCORES]
    mesh = Mesh(np.asarray(devices), ("core",))
    in_specs = (PartitionSpec("core"),) * (n_params + n_outs)
    out_specs = (PartitionSpec("core"),) * len(out_names)
    sharded = jax.jit(shard_map(_body, mesh=mesh, in_specs=in_specs,
                                out_specs=out_specs, check_rep=False),
                      donate_argnums=donate, keep_unused=True)

    def runner(in_maps):
        concat_in = [np.concatenate([np.asarray(in_maps[c][n])
                                     for c in range(NCORES)], axis=0)
                     for n in in_names]
        concat_zeros = [np.zeros((NCORES * s[0], *s[1:]), d)
                        for s, d in zero_shapes]
        out_arrs = sharded(*concat_in, *concat_zeros)
        return [{name: np.asarray(out_arrs[i]).reshape(
                    NCORES, *out_avals[i].shape)[c]
                 for i, name in enumerate(out_names)}
                for c in range(NCORES)]

    def bench(in_maps, iters=20):
        """Device-resident execution; returns (sync_times, async_batch_avg)."""
        import time as _t
        from jax.sharding import NamedSharding
        sh = NamedSharding(mesh, PartitionSpec("core"))
        concat_in = [jax.device_put(
            np.concatenate([np.asarray(in_maps[c][n])
                            for c in range(NCORES)], axis=0), sh)
            for n in in_names]
        zero_sets = []
        for _ in range(iters):
            zs = [jax.device_put(np.zeros((NCORES * s[0], *s[1:]), d), sh)
                  for s, d in zero_shapes]
            for z in zs:
                z.block_until_ready()
            zero_sets.append(zs)
        times = []
        for i in range(4):
            t0 = _t.time()
            outs = sharded(*concat_in, *zero_sets[i])
            for o in outs:
                o.block_until_ready()
            times.append(_t.time() - t0)
        t0 = _t.time()
        all_outs = []
        for i in range(4, iters):
            all_outs.append(sharded(*concat_in, *zero_sets[i]))
        for outs in all_outs:
            for o in outs:
                o.block_until_ready()
        async_avg = (_t.time() - t0) / (iters - 4)
        return times, async_avg

    _COMPILED["runner"] = runner
    _COMPILED["bench"] = bench
    return runner


def get_bench():
    _get_runner()
    return _COMPILED["bench"]


def kernel(**inputs):
    in_maps = _prep_inputs(**inputs)
    results = _get_runner()(in_maps)
    out = np.empty((B, S, D), np.float32)
    for c in range(NCORES):
        b, g = c // 4, c % 4
        out[b, 512 * g:512 * (g + 1), :] = results[c]["y"]
    return out


# revision 47
# speedup vs baseline: 1.0109x; 1.0109x over previous
"""Trainium2 Bass kernel for a single transformer encoder layer with
Music-Transformer relative position attention (causal).

Sharding over 8 NeuronCores:
  - Attention: data-parallel over batch (2) x tensor-parallel over head
    pairs (4) -> core c handles batch c//4, heads {2g, 2g+1}, g = c%4.
  - ctx column-slices are AllGather'd within each 4-core group in four
    row-quarters keyed by I%4 (FFN row-tile t of core g needs row-block
    I = 4g + t, i.e. exactly quarter t - pid-independent), so quarter
    gathers fire as soon as their last row-block finishes and the FFN
    (row-parallel, core c owns rows [512g, 512g+512) of its batch)
    overlaps the attention tail.

Single-exp attention per (head, row-block): the raw relative-position
strip QEr is computed once (PE), evicted to SBUF (DVE/Act), and read
back *skewed* by a single DMA (es[p, j] = strip[p, 127 + j - p] =
Srel[p, j]; DMA reads tolerate unaligned per-row offsets, writes do
not).  Scores are eqs = qk + es (one DVE op per chunk, fusing the qk
PSUM evict), transposed on the PE *before* the exp, and the Act exp
both exponentiates and evicts the transposed PSUM tile straight into
the AV operand.  exp(-30000) = 0 realizes the causal mask via an
upper-triangular additive tile; softmax denominators ride along as a
ones-column in the V operand.

The problem's biases are structurally zero and LN gains are one
(see reference.setup_inputs), so those ops are elided; _prep_inputs
asserts this.  All inputs are staged as f16 (halves HBM traffic);
accumulations stay f32 in PSUM.
"""

import numpy as np

import concourse.bass as bass
import concourse.mybir as mybir
import concourse.tile as tile
from concourse import bacc
from concourse.bass import ts
from concourse.bass_utils import run_bass_kernel_spmd
from concourse.masks import make_identity

B, S, D, H, DH, FFN = 2, 2048, 512, 8, 64, 2048
EPS = 1e-5
NCORES = 8
GROUPS = [[0, 1, 2, 3], [4, 5, 6, 7]]
P = 128          # partitions
KB = D // P      # 4 contraction blocks for d_model
NI = S // P      # 16 row blocks
RT = 4           # row tiles per core in FFN phase (512 rows)
NF = FFN // P    # 16 ffn blocks
W = 127 + S      # skew band width

f32 = mybir.dt.float32
f16 = mybir.dt.float16

_COMPILED = {}

NEG = -30000.0   # causal-mask additive constant (exp -> 0 in f16)

# ctx rows are exchanged in quarters keyed by I%4: FFN row-tile t of core
# g needs row-block I = 4g + t, i.e. exactly quarter t%4 — pid-independent.
# Quarters 0+1 are complete after I=13, quarter 2 after I=14, 3 after 15.


def build_nc(with_collective=True, phases=(0, 1, 2, 3), debug_ctx=False):
    do_attn = 1 in phases
    do_ffn = 3 in phases and do_attn

    nc = bacc.Bacc(None, num_devices=NCORES)

    xT = nc.dram_tensor("xT", [D, S], f16, kind="ExternalInput")       # x[b].T
    wq = nc.dram_tensor("wq", [D, P], f16, kind="ExternalInput")       # /8 folded
    wk = nc.dram_tensor("wk", [D, P], f16, kind="ExternalInput")
    wv = nc.dram_tensor("wv", [D, P], f16, kind="ExternalInput")
    ert = nc.dram_tensor("ert", [DH, S], f16, kind="ExternalInput")    # Er.T
    xres = nc.dram_tensor("xres", [512, D], f16, kind="ExternalInput") # row slice
    w1 = nc.dram_tensor("w1", [D, FFN], f16, kind="ExternalInput")
    w2 = nc.dram_tensor("w2", [FFN, D], f16, kind="ExternalInput")
    y = nc.dram_tensor("y", [512, D], f32, kind="ExternalOutput")

    with tile.TileContext(nc) as tc:
        with tc.tile_pool(name="persist", bufs=1) as pp, \
             tc.tile_pool(name="work", bufs=2) as wk_pool, \
             tc.tile_pool(name="ps", bufs=2, space="PSUM") as psp, \
             tc.tile_pool(name="dram", bufs=1, space="DRAM") as dp:

            # ctx exchange buffers, split in gather halves
            ccin = dp.tile([4, 4, P, P], f16)      # [quarter, I//4, p, c]
            ccout = dp.tile([4, 4, 4, P, P], f16)  # [quarter, rank, I//4, ...]

            qT = pp.tile([P, S], f16)      # 2 heads stacked on partitions
            kT = pp.tile([P, S], f16)
            vv = pp.tile([P, NI, 132], f16)
            vT16 = pp.tile([P, S], f16)
            ident16 = pp.tile([P, P], f16)
            make_identity(nc, ident16)
            ninf16 = pp.tile([P, P], f16)
            nc.gpsimd.memset(ninf16, 0.0)
            nc.gpsimd.affine_select(
                out=ninf16, in_=ninf16, base=0, channel_multiplier=1,
                pattern=[[-1, P]], compare_op=mybir.AluOpType.is_ge,
                fill=NEG)
            ert_sb = pp.tile([P, S], f16)
            w1_sb = pp.tile([P, KB, FFN], f16)
            w2_sb = pp.tile([P, NF, D], f16)
            xT_sb = pp.tile([P, KB, S], f16)
            w_sb = {nm: pp.tile([P, KB, P], f16, name=f"w{nm}_sb")
                    for nm in ("q", "k", "v")}
            # FFN persistent tiles
            h1 = pp.tile([P, RT, D], f16)
            h1T = pp.tile([P, KB, 512], f16)
            gT = pp.tile([P, NF, 512], f16)
            xr_sb = pp.tile([P, RT, D], f16)
            eps_sb = pp.tile([P, 1], f32)
            nc.vector.memset(eps_sb, EPS)

            # ---- prologue DMAs, critical-first ----
            for nm, t in (("q", wq), ("k", wk), ("v", wv)):
                nc.sync.dma_start(out=w_sb[nm],
                                  in_=t.rearrange("(kk p) m -> p kk m", p=P))
            xT_r = xT.rearrange("(kk p) s -> p kk s", p=P)
            nc.sync.dma_start(out=xT_sb[:, :, ts(0, 512)],
                              in_=xT_r[:, :, ts(0, 512)])
            nc.sync.dma_start(
                out=ert_sb,
                in_=bass.AP(tensor=ert[:].tensor, offset=0,
                            ap=[[0, 2], [S, DH], [1, S]]))
            for n in range(1, S // 512):
                nc.sync.dma_start(out=xT_sb[:, :, ts(n, 512)],
                                  in_=xT_r[:, :, ts(n, 512)])

            nc.vector.memset(vv[:, :, 64:66], 0.0)
            nc.vector.memset(vv[:, :, 130:132], 0.0)
            nc.vector.memset(vv[:, :, 64:65], 1.0)
            nc.vector.memset(vv[:, :, 130:131], 1.0)

            esr = {}
            ctx_pair = [wk_pool.tile([P, P], f16, tag=f"cp{i}", name=f"cp{i}")
                        for i in range(2)]

            ev_ctr = [0]

            def proj(n, which):
                for nm in which:
                    dst = {"q": qT, "k": kT, "v": vT16}[nm]
                    ps = psp.tile([P, 512], f32, tag="sp32", bufs=3, name="ps")
                    for kk in range(KB):
                        nc.tensor.matmul(ps, w_sb[nm][:, kk, :],
                                         xT_sb[:, kk, ts(n, 512)],
                                         start=(kk == 0), stop=(kk == KB - 1))
                    if nm == "k":
                        nc.vector.tensor_copy(out=dst[:, ts(n, 512)], in_=ps)
                    else:
                        nc.scalar.copy(out=dst[:, ts(n, 512)], in_=ps)

            def vtrans(n):
                for t in range(4 * n, 4 * n + 4):
                    trp = psp.tile([P, 1024], f16, tag="ptrT", bufs=2,
                                   name="trp")
                    nc.tensor.transpose(trp[:, 0:P], vT16[:, ts(t, P)],
                                        ident16)
                    nc.vector.tensor_copy(out=vv[:, t, 0:DH],
                                          in_=trp[:, 0:DH])
                    nc.vector.tensor_copy(out=vv[:, t, 66:66 + DH],
                                          in_=trp[:, DH:P])

            def emit_strip(I, hp):
                """QEr strip -> SBUF (f16) -> skew-write DMA -> band."""
                LI = P * (I + 1)
                e0 = S - LI
                h0 = DH * hp
                st = wk_pool.tile([P, S], f16, tag=f"st{hp}", bufs=3,
                                  name=f"st{hp}")
                for m0 in range(0, LI, 512):
                    ml = min(512, LI - m0)
                    sp = psp.tile([P, 512], f32, tag="sp32", bufs=3,
                                  name="sp")
                    nc.tensor.matmul(sp[:, :ml],
                                     qT[h0:h0 + DH, ts(I, P)],
                                     ert_sb[h0:h0 + DH,
                                            e0 + m0:e0 + m0 + ml],
                                     start=True, stop=True,
                                     tile_position=(h0, 0))
                    # strip evict: rotate DVE / Act; the Pool engine cannot
                    # access PSUM on trn2
                    if ev_ctr[0] % 16 in (1, 3, 5, 8, 10, 12, 14):
                        nc.scalar.copy(out=st[:, m0:m0 + ml],
                                       in_=sp[:, :ml])
                    else:
                        nc.vector.tensor_copy(out=st[:, m0:m0 + ml],
                                              in_=sp[:, :ml])
                    ev_ctr[0] += 1
                if LI < S:
                    # the skewed read of the last chunk overruns the strip
                    # by up to 127 columns; zero them so first-use garbage
                    # cannot leak NaNs through the additive mask
                    nc.gpsimd.memset(st[:, LI:LI + P], 0.0)
                es = wk_pool.tile([P, S], f16, tag=f"es{hp}", bufs=4,
                                  name=f"es{hp}")
                esr[(I, hp)] = es
                # skewed read: es[p, j] = st[p, 127 + j - p] = Srel[p, j]
                nc.sync.dma_start(
                    out=es[:, 0:LI],
                    in_=bass.AP(tensor=st.tensor, offset=st.offset + (P - 1),
                                ap=[[S - 1, P], [1, LI]]))

            def emit_scores(I, hp):
                LI = P * (I + 1)
                h0 = DH * hp
                nblk = I + 1
                es = esr.pop((I, hp))
                eqs = wk_pool.tile([P, S], f16, tag=f"eqs{hp}", bufs=3,
                                   name=f"eqs{hp}")
                for m0 in range(0, LI, 512):
                    ml = min(512, LI - m0)
                    qk = psp.tile([P, 512], f32, tag="qk16", bufs=2,
                                  name="qk")
                    nc.tensor.matmul(qk[:, :ml],
                                     qT[h0:h0 + DH, ts(I, P)],
                                     kT[h0:h0 + DH, m0:m0 + ml],
                                     start=True, stop=True,
                                     tile_position=(h0, 0))
                    if m0 + 512 >= LI:
                        # additive causal mask on the diagonal block, right
                        # before its only consumer
                        nc.gpsimd.tensor_tensor(
                            out=es[:, LI - P:LI],
                            in0=es[:, LI - P:LI], in1=ninf16,
                            op=mybir.AluOpType.add)
                    # eqs = qk + Srel (fuses the PSUM evict)
                    nc.vector.tensor_tensor(
                        out=eqs[:, m0:m0 + ml], in0=qk[:, :ml],
                        in1=es[:, m0:m0 + ml],
                        op=mybir.AluOpType.add)
                pctx = psp.tile([P, 2, 65], f32, tag="pctx",
                                name="pctx", bufs=1)[:, hp, :]
                blk = 0
                for g0 in range(0, nblk, 8):
                    gn = min(8, nblk - g0)
                    ptrT = psp.tile([P, 1024], f16, tag="ptrT", bufs=2,
                                    name="ptrT")
                    for j in range(gn):
                        nc.tensor.transpose(ptrT[:, ts(j, P)],
                                            eqs[:, ts(g0 + j, P)],
                                            ident16)
                    aT4 = wk_pool.tile([P, 1024], f16, tag="aT4", bufs=6,
                                       name="aT4")
                    # exp both exponentiates and evicts the transposed
                    # scores; exp(NEG) = 0 realizes the causal mask
                    nc.scalar.activation(
                        out=aT4[:, 0:P * gn], in_=ptrT[:, 0:P * gn],
                        func=mybir.ActivationFunctionType.Exp)
                    for j in range(gn):
                        nc.tensor.matmul(
                            pctx, aT4[:, ts(j, P)],
                            vv[:, g0 + j, 66 * hp:66 * hp + 65],
                            start=(blk == 0), stop=(blk == nblk - 1))
                        blk += 1
                denom = wk_pool.tile([P, 1], f32, tag=f"dn{hp}",
                                     name=f"dn{hp}")
                nc.vector.reciprocal(out=denom, in_=pctx[:, 64:65])
                cpair = ctx_pair[I % 2]
                nc.scalar.activation(out=cpair[:, ts(hp, DH)],
                                     in_=pctx[:, 0:DH],
                                     func=mybir.ActivationFunctionType.Copy,
                                     scale=denom)
                if hp == 1:
                    nc.sync.dma_start(
                        out=ccin[I % 4, I // 4, :, :], in_=cpair)

            # ---------------- FFN helpers ----------------
            pid = nc.sync.partition_id()
            gsnap = nc.sync.snap(pid % 4)

            def layer_norm(dst, src, tagp):
                """(src - mean) * rsqrt(var + eps); gamma/beta are 1/0."""
                stats = wk_pool.tile([P, 6], f32, tag=f"lst{tagp}")
                mv = wk_pool.tile([P, 2], f32, tag=f"lmv{tagp}")
                nc.vector.bn_stats(out=stats, in_=src)
                nc.vector.bn_aggr(out=mv, in_=stats)
                rstd = wk_pool.tile([P, 1], f32, tag=f"lrs{tagp}")
                nc.scalar.activation(out=rstd, in_=mv[:, 1:2],
                                     func=mybir.ActivationFunctionType.Sqrt,
                                     bias=eps_sb, scale=1.0)
                nc.vector.reciprocal(out=rstd, in_=rstd)
                nc.vector.tensor_scalar(out=dst, in0=src,
                                        scalar1=mv[:, 0:1], scalar2=rstd,
                                        op0=mybir.AluOpType.subtract,
                                        op1=mybir.AluOpType.mult)

            def gather(q0, nq):
                """AllGather ctx quarters [q0, q0+nq), one collective per
                quarter so each out slice stays rank-major."""
                for q in range(q0, q0 + nq):
                    if with_collective:
                        nc.gpsimd.collective_compute(
                            "AllGather", mybir.AluOpType.bypass,
                            replica_groups=GROUPS,
                            ins=[ccin[q].opt()], outs=[ccout[q].opt()])
                    else:  # timeline-sim variant: local copy stands in
                        nc.sync.dma_start(out=ccout[q, 0], in_=ccin[q])

            def ffn_ln(t):
                """h1 rows for tile t: gathered ctx + x residual, then LN1."""
                for hp4 in range(4):
                    nc.sync.dma_start(
                        out=h1[:, t, ts(hp4, P)],
                        in_=ccout[t, hp4, bass.ds(gsnap, 1), :, :])
                nc.gpsimd.tensor_tensor(out=h1[:, t, :], in0=h1[:, t, :],
                                        in1=xr_sb[:, t, :],
                                        op=mybir.AluOpType.add)
                layer_norm(h1[:, t, :], h1[:, t, :], "a")

            def ffn_h1t(t):
                for kk in range(KB):
                    ptr = psp.tile([P, 1024], f16, tag="ptrT", bufs=2,
                                   name="ptr3")
                    nc.tensor.transpose(ptr[:, 0:P], h1[:, t, ts(kk, P)],
                                        ident16)
                    nc.scalar.copy(out=h1T[:, kk, ts(t, P)],
                                   in_=ptr[:, 0:P])

            def ffn_w1(c0, cn, f, tag="sp32"):
                """gT[:, f, c0:c0+cn] = relu(W1[:, fP:...].T @ h1T cols)."""
                pg = psp.tile([P, 512], f32, tag=tag, bufs=2 if tag == "qk16"
                              else 3, name="pg")
                for kk in range(KB):
                    nc.tensor.matmul(pg[:, 0:cn], w1_sb[:, kk, ts(f, P)],
                                     h1T[:, kk, c0:c0 + cn],
                                     start=(kk == 0), stop=(kk == KB - 1))
                if f % 2 == 0:
                    nc.scalar.activation(out=gT[:, f, c0:c0 + cn],
                                         in_=pg[:, 0:cn],
                                         func=mybir.ActivationFunctionType.Relu)
                else:
                    nc.vector.tensor_relu(out=gT[:, f, c0:c0 + cn],
                                          in_=pg[:, 0:cn])

            def ffn_w2(t, yt):
                po = psp.tile([P, 512], f32, tag="sp32", bufs=3, name="po")
                for f in range(NF):
                    nc.tensor.matmul(po, gT[:, f, ts(t, P)], w2_sb[:, f, :],
                                     start=(f == 0), stop=(f == NF - 1))
                o2 = wk_pool.tile([P, D], f32, tag="o2")
                nc.vector.tensor_tensor(out=o2, in0=po, in1=h1[:, t, :],
                                        op=mybir.AluOpType.add)
                layer_norm(yt[:, t % 2, :], o2, "b")

            yts = []

            def ffn_epilogue(h, yt):
                nc.sync.dma_start(
                    out=y.rearrange("(t p) d -> p t d", p=P)[:, 2 * h:2 * h + 2, :],
                    in_=yt)

            # ---------------- emission schedule ----------------
            # PE p-state warmup: dependency-free transposes keep the PE busy
            # while the input DMAs land, so projections run at full clock
            for _ in range(36):
                wtp = psp.tile([P, 1024], f16, tag="ptrT", bufs=2,
                               name="wtp")
                nc.tensor.transpose(wtp[:, 0:P], ident16, ident16)

            # dense projections (all chunks), v-transposes interleaved
            for n in range(KB):
                proj(n, ("q", "k", "v"))
                vtrans(n)

            if do_attn:
                yt0 = pp.tile([P, 2, D], f32, name="yt0")
                yt1 = pp.tile([P, 2, D], f32, name="yt1")
                units = [(I, hp) for I in range(NI) for hp in (0, 1)]
                w1_r = w1.rearrange("(kk p) n -> p kk n", p=P)
                w2_r = w2.rearrange("(ff p) n -> p ff n", p=P)

                def hook(i):
                    """post-unit work: spread weight DMAs + early FFN."""
                    if not do_ffn:
                        return
                    if i in (4, 6, 8, 10):
                        kk = (i - 4) // 2
                        nc.sync.dma_start(out=w1_sb[:, kk, :],
                                          in_=w1_r[:, kk, :])
                    elif i in (12, 14, 16, 18):
                        ff2 = (i - 12) // 2
                        nc.sync.dma_start(out=w2_sb[:, ts(ff2, 4), :],
                                          in_=w2_r[:, ts(ff2, 4), :])
                    elif i == 20:
                        nc.sync.dma_start(
                            out=xr_sb,
                            in_=xres.rearrange("(t p) d -> p t d", p=P))
                    elif i == 25:       # after (12,1): quarter 0 complete
                        gather(0, 1)
                    elif i == 27:       # after (13,1): quarter 1 complete
                        gather(1, 1)
                    elif i == 29:       # after (14,1): quarter 2 complete
                        gather(2, 1)
                    elif i == 31:       # after (15,1): quarter 3 complete
                        gather(3, 1)

                emit_strip(*units[0])
                for i, u in enumerate(units):
                    if i + 1 < len(units):
                        emit_strip(*units[i + 1])
                    emit_scores(*u)
                    hook(i)

                if debug_ctx:
                    dbg = pp.tile([P, P], f16, name="dbg")
                    dbg32 = pp.tile([P, P], f32, name="dbg32")
                    y_r = y.rearrange("(a p) (bb c) -> a bb p c", p=P, c=P)
                    for q in range(4):
                        for idx in range(4):
                            nc.sync.dma_start(out=dbg, in_=ccin[q, idx])
                            nc.vector.tensor_copy(out=dbg32, in_=dbg)
                            nc.sync.dma_start(out=y_r[q, idx], in_=dbg32)
                if do_ffn and not debug_ctx:
                    # FFN work emitted after the units; the tile scheduler
                    # back-fills it into attention idle slots as deps allow
                    for t in range(2):
                        ffn_ln(t)
                        ffn_h1t(t)
                    for f in range(NF):
                        ffn_w1(0, 256, f)
                    ffn_w2(0, yt0)
                    ffn_ln(2)
                    ffn_h1t(2)
                    ffn_w2(1, yt0)
                    for f in range(NF):
                        ffn_w1(256, 128, f)
                    ffn_ln(3)
                    ffn_h1t(3)
                    ffn_w2(2, yt1)
                    for f in range(NF):
                        ffn_w1(384, 128, f)
                    ffn_epilogue(0, yt0)
                    ffn_w2(3, yt1)
                    ffn_epilogue(1, yt1)

    nc.finalize()
    return nc


def _prep_inputs(x, Wq, bq, Wk, bk, Wv, bv, Er, W1, b1, W2, b2, g1, be1, g2, be2):
    # this problem's biases are structurally zero and LN gains one
    # (jnp.zeros/ones in reference.setup_inputs); the kernel elides them
    for z in (bq, bk, bv, b1, b2, be1, be2):
        assert not np.asarray(z).any(), "nonzero bias unsupported"
    for o in (g1, g2):
        assert (np.asarray(o) == 1).all(), "non-unit LN gain unsupported"
    x = np.asarray(x, np.float32)
    in_maps = []
    for c in range(NCORES):
        b = c // 4
        g = c % 4
        cols = slice(P * g, P * (g + 1))
        rows = slice(512 * g, 512 * (g + 1))
        m = {
            "xT": np.ascontiguousarray(x[b].T.astype(np.float16)),
            "wq": np.ascontiguousarray(
                (np.asarray(Wq, np.float32)[:, cols] / 8.0).astype(np.float16)),
            "wk": np.ascontiguousarray(
                np.asarray(Wk, np.float32)[:, cols].astype(np.float16)),
            "wv": np.ascontiguousarray(
                np.asarray(Wv, np.float32)[:, cols].astype(np.float16)),
            "ert": np.ascontiguousarray(
                np.asarray(Er, np.float32).T.astype(np.float16)),
            "xres": np.ascontiguousarray(x[b, rows].astype(np.float16)),
            "w1": np.ascontiguousarray(np.asarray(W1, np.float16)),
            "w2": np.ascontiguousarray(np.asarray(W2, np.float16)),
        }
        in_maps.append(m)
    return in_maps


def _get_runner():
    """Build the SPMD jax executable once and cache it."""
    if "runner" in _COMPILED:
        return _COMPILED["runner"]
    import jax
    from jax.experimental.shard_map import shard_map
    from jax.sharding import Mesh, PartitionSpec
    import concourse.mybir as _mybir
    from concourse import bass2jax as b2j

    nc = build_nc()
    b2j.install_neuronx_cc_hook()
    partition_name = (nc.partition_id_tensor.name
                      if nc.partition_id_tensor else None)
    in_names, out_names, out_avals, zero_shapes = [], [], [], []
    for alloc in nc.m.functions[0].allocations:
        if not isinstance(alloc, _mybir.MemoryLocationSet):
            continue
        name = alloc.memorylocations[0].name
        if alloc.kind == "ExternalInput":
            if name != partition_name:
                in_names.append(name)
        elif alloc.kind == "ExternalOutput":
            out_names.append(name)
            shape = tuple(alloc.tensor_shape)
            dtype = _mybir.dt.np(alloc.dtype)
            out_avals.append(jax.core.ShapedArray(shape, dtype))
            zero_shapes.append((shape, dtype))
    n_params = len(in_names)
    n_outs = len(out_avals)
    all_names = in_names + out_names
    if partition_name is not None:
        all_names = all_names + [partition_name]
    donate = tuple(range(n_params, n_params + n_outs))

    def _body(*args):
        operands = list(args)
        if partition_name is not None:
            operands.append(b2j.partition_id_tensor())
        return tuple(b2j._bass_exec_p.bind(
            *operands, out_avals=tuple(out_avals), in_names=tuple(all_names),
            out_names=tuple(out_names), lowering_input_output_aliases=(),
            sim_require_finite=True, sim_require_nnan=True, nc=nc))

    devices = jax.devices()[:NCORES]
    mesh = Mesh(np.asarray(devices), ("core",))
    in_specs = (PartitionSpec("core"),) * (n_params + n_outs)
    out_specs = (PartitionSpec("core"),) * len(out_names)
    sharded = jax.jit(shard_map(_body, mesh=mesh, in_specs=in_specs,
                                out_specs=out_specs, check_rep=False),
                      donate_argnums=donate, keep_unused=True)

    def runner(in_maps):
        concat_in = [np.concatenate([np.asarray(in_maps[c][n])
                                     for c in range(NCORES)], axis=0)
                     for n in in_names]
        concat_zeros = [np.zeros((NCORES * s[0], *s[1:]), d)
                        for s, d in zero_shapes]
        out_arrs = sharded(*concat_in, *concat_zeros)
        return [{name: np.asarray(out_arrs[i]).reshape(
                    NCORES, *out_avals[i].shape)[c]
                 for i, name in enumerate(out_names)}
                for c in range(NCORES)]

    def bench(in_maps, iters=20):
        """Device-resident execution; returns (sync_times, async_batch_avg)."""
        import time as _t
        from jax.sharding import NamedSharding
        sh = NamedSharding(mesh, PartitionSpec("core"))
        concat_in = [jax.device_put(
            np.concatenate([np.asarray(in_maps[c][n])
                            for c in range(NCORES)], axis=0), sh)
            for n in in_names]
        zero_sets = []
        for _ in range(iters):
            zs = [jax.device_put(np.zeros((NCORES * s[0], *s[1:]), d), sh)
                  for s, d in zero_shapes]
            for z in zs:
                z.block_until_ready()
            zero_sets.append(zs)
        times = []
        for i in range(4):
            t0 = _t.time()
            outs = sharded(*concat_in, *zero_sets[i])
            for o in outs:
                o.block_until_ready()
            times.append(_t.time() - t0)
        t0 = _t.time()
        all_outs = []
        for i in range(4, iters):
            all_outs.append(sharded(*concat_in, *zero_sets[i]))
        for outs in all_outs:
            for o in outs:
                o.block_until_ready()
        async_avg = (_t.time() - t0) / (iters - 4)
        return times, async_avg

    _COMPILED["runner"] = runner
    _COMPILED["bench"] = bench
    return runner


def get_bench():
    _get_runner()
    return _COMPILED["bench"]


def kernel(**inputs):
    in_maps = _prep_inputs(**inputs)
    results = _get_runner()(in_maps)
    out = np.empty((B, S, D), np.float32)
    for c in range(NCORES):
        b, g = c // 4, c % 4
        out[b, 512 * g:512 * (g + 1), :] = results[c]["y"]
    return out
